# revision 43
# baseline (speedup 1.0000x reference)
"""Causal self-attention Trainium2 kernel (B=4, T=2048, E=1024, H=16, D=64).

Sharding: 8 cores = batch(4) x head-group(2). Each core computes the full
attention for 8 heads of one batch element plus its half of the output
projection; the host sums the two out-proj partials per batch element.

v3 dataflow (per core, all matmul operands bf16, PSUM f32):
  - x^T [E,T] lives fully in SBUF, loaded with 4 large DMAs (one per
    T-quarter) on two queues so the first V-proj chain starts ~4us in.
  - Projection chains (V then QK per quarter) are interleaved into the
    attention stream as fillers so the PE never idles while ScalarE works
    through the exps; out-projection row tiles become fillers as soon as
    their query quarter is normalized.
  - S^T chunks [128kv, 2x512q]: two heads of a pair issued as row-group
    tiled matmul pairs (tile_position (0,0)/(64,0)) running concurrently.
    Diagonal chunks only compute live columns; exp on ScalarE -> bf16;
    causal mask via DVE multiply with a [128,640] zeros|tril constant.
  - y^T accumulation [65,512] per head, lhsT = V_aug (ones column carries
    the softmax denominator through the PV matmul).
  - Drain per (pair, tile): DVE casts y rows to yt/tmpb, tmpb is DMA'd
    into yt's upper partitions immediately (not norm-gated), DVE
    reciprocal runs directly on the PSUM denominator rows (partition 64),
    one cast + one tiny DMA lands both rcp rows on a partition-0 table.
  - Norm per (pair, tile), emitted ~one block later so every op's deps are
    met when it reaches its engine FIFO: 2 gpsimd partition_broadcasts +
    2 in-place DVE multiplies on yt. No cross-engine convoys.
  - ScalarE exp table is pre-warmed during the prologue DMAs.
"""

import numpy as np
import ml_dtypes

import concourse.bass as bass
import concourse.bacc as bacc
import concourse.mybir as mybir
import concourse.tile as tile
from concourse import bass_utils

f32 = mybir.dt.float32
bf16 = mybir.dt.bfloat16
FP = mybir.dt.float32  # psum dtype

P = 128
B, T, E = 4, 2048, 1024
H, D = 16, 64
HPC = H // 2            # heads per core = 8
NE = E // P             # 8 e-chunks
NTT = T // P            # 16 kv chunks
NQ = T // 512           # 4 query tiles of 512
SCALE = 1.0 / np.sqrt(D)

Exp = mybir.ActivationFunctionType.Exp
MULT = mybir.AluOpType.mult
IS_GE = mybir.AluOpType.is_ge

_CACHE = {}


def build(**opts):
    nc = bacc.Bacc("TRN2", target_bir_lowering=False, debug=False, num_devices=8)

    xT_d = nc.dram_tensor("xT", [P, NE, T], bf16, kind="ExternalInput")
    wvP_d = nc.dram_tensor("wvP", [P, NE, 512], bf16, kind="ExternalInput")
    wqkP_d = nc.dram_tensor("wqkP", [P, 8, NE, P], bf16, kind="ExternalInput")
    woP_d = nc.dram_tensor("woP", [P, 4, E], bf16, kind="ExternalInput")
    mask_d = nc.dram_tensor("mask", [P, P], bf16, kind="ExternalInput")
    out_d = nc.dram_tensor("out", [T, E], bf16, kind="ExternalOutput")
    dbgL_d = dbgB_d = None
    if opts.pop("debug_rcp", False):
        dbgL_d = nc.dram_tensor("dbgL", [16, 1024], f32,
                                kind="ExternalOutput")
        dbgB_d = nc.dram_tensor("dbgB", [16, 1024], bf16,
                                kind="ExternalOutput")

    with tile.TileContext(nc) as tc:
        build_body(tc, xT_d, wvP_d, wqkP_d, woP_d, mask_d, out_d,
                   dbgL_d=dbgL_d, dbgB_d=dbgB_d, **opts)
    nc.compile()
    return nc


def build_body(tc, xT_d, wvP_d, wqkP_d, woP_d, mask_d, out_d,
               pss_bufs=2, psy_bufs=2, norm_mode="full",
               dbgL_d=None, dbgB_d=None):
    nc = tc.nc

    from contextlib import ExitStack
    with ExitStack() as top:
        per = top.enter_context(tc.tile_pool(name="per", bufs=1))

        qk_sb = per.tile([P, 8, T], bf16)            # chunks 0-3: Q^T, 4-7: K^T
        v_sb = per.tile([P, NTT, HPC, D + 1], bf16)  # [kv_p, kv_chunk, head, d|1]
        yt_sb = per.tile([P, 4, T], bf16)            # [f%128, f//128, q]
        x_sb = per.tile([P, NE, T], bf16)            # x^T resident [p, e, t]
        wv_sb = per.tile([P, NE, 512], bf16)         # V-proj weights
        wqk_sb = per.tile([P, 8, NE, P], bf16)       # QK-proj weights per f-chunk
        wo_sb = per.tile([P, 4, E], bf16)            # out-proj weights
        mask_sb = per.tile([P, P], bf16)             # tril(128).T
        warm_sb = per.tile([4, 512], bf16)           # gpsimd ucode warmup dst
        # self-managed ptt ring: stale regions are never read (exp writes
        # [q0:512] per head and PV streams only those columns)
        ptt_ring = [per.tile([P, 1024], bf16, name=f"ptt{k}")
                    for k in range(4)]
        ptt_ctr = [0]

        # --------- prologue DMAs: 4 big x loads + weights, spread across
        # queues; warm the gpsimd ucode library and the ScalarE exp table
        # during the transfer ----------
        nc.gpsimd.partition_broadcast(warm_sb, ptt_ring[1][0:1, 0:512])
        # Priority-ordered prologue: the head is HBM-bound on 7MB of
        # input, but the first V chain needs only x-q0 + wv (2MB). Those
        # go first on the two HW queues; wqk pairs follow in the order
        # the attention needs them (host pre-interleaves ft as
        # 0,4,1,5,2,6,3,7); x quarters 2-3 are triggered later (their
        # dma_starts are emitted after quarter-0 V finishes, see below).
        nc.sync.dma_start(x_sb[:, :, 0:512], xT_d[:, :, 0:512])
        nc.scalar.dma_start(wv_sb, wvP_d[:, :, :])
        nc.sync.dma_start(wqk_sb[:, 0:2], wqkP_d[:, 0:2])
        nc.scalar.dma_start(wqk_sb[:, 2:4], wqkP_d[:, 2:4])
        nc.sync.dma_start(x_sb[:, :, 512:1024], xT_d[:, :, 512:1024])
        nc.scalar.dma_start(wqk_sb[:, 4:6], wqkP_d[:, 4:6])
        nc.scalar.dma_start(wqk_sb[:, 6:8], wqkP_d[:, 6:8])
        nc.scalar.dma_start(wo_sb, woP_d[:, :, :])
        nc.sync.dma_start(mask_sb, mask_d[:, :])
        # warm the exp table-set while DMAs fly (first ACT pays ~2.7us)
        nc.scalar.activation(warm_sb[:, 0:8], warm_sb[:, 0:8],
                             Exp, scale=1.0)
        # logical ft -> position in the host-interleaved wqk layout
        ftpos = {0: 0, 4: 1, 1: 2, 5: 3, 2: 4, 6: 5, 3: 6, 7: 7}

        # pool creation order: psp LAST so it sits on top of the PSUM stack
        # and can be swapped for the out-proj pool after projections end
        drn = top.enter_context(tc.tile_pool(name="drn", bufs=2))
        nrm = top.enter_context(tc.tile_pool(name="nrm", bufs=3))
        ost = top.enter_context(tc.tile_pool(name="ost", bufs=2))
        pss = top.enter_context(
            tc.tile_pool(name="pss", bufs=pss_bufs, space="PSUM"))
        psy = top.enter_context(
            tc.tile_pool(name="psy", bufs=psy_bufs, space="PSUM"))
        psp_ctx = ExitStack()
        psp = psp_ctx.enter_context(
            tc.tile_pool(name="psp", bufs=1, space="PSUM"))
        pools = {}
        done = set()
        lps = {}

        def piece_v(th, tti):
            # one V-projection chain: v_sb chunk tt, natural layout
            if ("v", th, tti) in done:
                return
            done.add(("v", th, tti))
            tt = th * 4 + tti
            ps = psp.tile([P, 1024], FP, tag="pq")
            for e in range(NE):
                nc.tensor.matmul(
                    ps[:, 0:512],
                    lhsT=x_sb[:, e, tt * P:(tt + 1) * P],
                    rhs=wv_sb[:, e, :],
                    start=(e == 0), stop=(e == NE - 1))
            nc.vector.tensor_copy(
                v_sb[:, tt, :, 0:D],
                ps[:, 0:512].rearrange("p (h d) -> p h d", h=HPC))
            if tti == 3:
                # ones column for this quarter (never keeps in_: cond<0)
                ov = v_sb[:, th * 4:(th + 1) * 4, :, D:D + 1]
                iv = v_sb[:, th * 4:(th + 1) * 4, :, 0:1]
                nc.gpsimd.affine_select(
                    ov, iv, pattern=[[0, 4], [0, HPC], [0, 1]],
                    compare_op=IS_GE, fill=1.0, base=-1,
                    channel_multiplier=0)

        def piece_qk(hf, ft):
            # one QK-projection chain over a T-half: each weight load
            # feeds two N=512 matmuls (adjacent quarters, same lhsT)
            if ("qk", hf, ft) in done:
                return
            done.add(("qk", hf, ft))
            h0 = hf * 1024
            ps = psp.tile([P, 1024], FP, tag="pq")
            for e in range(NE):
                nc.tensor.matmul(
                    ps[:, 0:512],
                    lhsT=wqk_sb[:, ftpos[ft], e, :],
                    rhs=x_sb[:, e, h0:h0 + 512],
                    start=(e == 0), stop=(e == NE - 1))
                nc.tensor.matmul(
                    ps[:, 512:1024],
                    lhsT=wqk_sb[:, ftpos[ft], e, :],
                    rhs=x_sb[:, e, h0 + 512:h0 + 1024],
                    start=(e == 0), stop=(e == NE - 1))
            nc.vector.tensor_copy(
                qk_sb[:, ft, h0:h0 + 1024], ps)

        def piece_out(tt):
            # one out-proj row tile, both halves: each yt weight load
            # feeds two N=512 matmuls into the two po PSUM buffers
            pso = pools["pso"]
            po0 = pso.tile([P, 512], FP, tag="po", name=f"po0_{tt}")
            po1 = pso.tile([P, 512], FP, tag="po", name=f"po1_{tt}")
            for c2 in range(4):
                nc.tensor.matmul(
                    po0,
                    lhsT=yt_sb[:, c2, tt * P:(tt + 1) * P],
                    rhs=wo_sb[:, c2, 0:512],
                    start=(c2 == 0), stop=(c2 == 3))
                nc.tensor.matmul(
                    po1,
                    lhsT=yt_sb[:, c2, tt * P:(tt + 1) * P],
                    rhs=wo_sb[:, c2, 512:1024],
                    start=(c2 == 0), stop=(c2 == 3))
            st = ost.tile([P, 1024], bf16, tag="st")
            nc.vector.tensor_copy(st[:, 0:512], po0)
            nc.vector.tensor_copy(st[:, 512:1024], po1)
            nc.sync.dma_start(out_d[tt * P:(tt + 1) * P, :], st)

        def att_block(c, j, fillers=None, every=4):
            """Attention for head pair (2c, 2c+1), query tile j.

            Pops one filler (independent work) every `every` chunks so the
            PE stays busy while ScalarE works through the exps."""
            jsl = slice(j * 512, (j + 1) * 512)
            nkv = 4 * j + 4
            yps = [psy.tile([D + 1, 512], FP, tag="y", name=f"yps_{c}_{j}_{k}")
                   for k in range(2)]
            for i in range(nkv):
                off = i - 4 * j
                q0 = max(0, 128 * off)
                spt = pss.tile([P, 1024], FP, tag="s")
                for hh in range(2):
                    p0 = 64 * hh
                    nc.tensor.matmul(
                        spt[:, hh * 512 + q0:(hh + 1) * 512],
                        lhsT=qk_sb[p0:p0 + 64, 4 + c, i * P:(i + 1) * P],
                        rhs=qk_sb[p0:p0 + 64, c, j * 512 + q0:(j + 1) * 512],
                        start=True, stop=True,
                        tile_position=(p0, 0))
                ptt = ptt_ring[ptt_ctr[0] % 4]
                ptt_ctr[0] += 1
                if off < 0:
                    nc.scalar.activation(ptt, spt, Exp, scale=float(SCALE))
                else:
                    pv = ptt.rearrange("p (h q) -> p h q", h=2)
                    sv = spt.rearrange("p (h q) -> p h q", h=2)
                    nc.scalar.activation(pv[:, :, q0:512], sv[:, :, q0:512],
                                         Exp, scale=float(SCALE))
                    # causal mask: zero the dead upper triangle of the
                    # [q0,q0+128) block; columns below q0 are never read
                    for hx in range(2):
                        nc.vector.tensor_tensor(
                            pv[:, hx, q0:q0 + P], pv[:, hx, q0:q0 + P],
                            mask_sb, MULT)
                for hh in range(2):
                    # stream only the live columns [q0:512]; dead columns
                    # keep earlier (sub-diagonal) partial sums, which is
                    # exact since dead chunks contribute zero there
                    nc.tensor.matmul(
                        yps[hh][:, q0:512],
                        lhsT=v_sb[:, i, 2 * c + hh, :],
                        rhs=ptt[:, hh * 512 + q0:(hh + 1) * 512],
                        start=(i == 0), stop=(i == nkv - 1),
                        skip_group_check=True)
                if fillers and (i + 1) % every == 0 and fillers:
                    fillers.pop(0)()
            # drain: denominator path first (it gates the norm chain),
            # then the y-row casts. Rows are staged to partition 64 of
            # stg and DMA-shifted to a partition-0 tile for the norm
            # (custom-DVE/gpsimd ops mishandle nonzero base partitions).
            stg = drn.tile([D + 1, 1024], f32, tag="stg")
            nc.vector.tensor_copy(stg[D:D + 1, 0:512], yps[0][D:D + 1, :])
            nc.vector.tensor_copy(stg[D:D + 1, 512:1024], yps[1][D:D + 1, :])
            lp = nrm.tile([1, 1024], f32, tag="lp", bufs=4,
                          name=f"lp_{c}_{j}")
            nc.gpsimd.dma_start(lp, stg[D:D + 1, :])
            lps[(c, j)] = lp
            nc.vector.tensor_copy(yt_sb[0:D, c, jsl], yps[0][0:D, :])
            tmpb = drn.tile([D, 512], bf16, tag="tmpb", bufs=3)
            nc.vector.tensor_copy(tmpb, yps[1][0:D, :])
            nc.gpsimd.dma_start(yt_sb[64:128, c, jsl], tmpb)
            if dbgL_d is not None:
                nc.sync.dma_start(dbgL_d[4 * j + c], stg[D:D + 1, :])

        def norm(c, j, eng=None):
            """Normalize yt tile (c, j): two partition_broadcasts of the
            rcp rows + two in-place multiplies. Emitted ~a block after the
            drain so every dep is satisfied on arrival."""
            if norm_mode == "skip":
                return
            eng = eng or nc.vector
            jsl = slice(j * 512, (j + 1) * 512)
            lp = lps.pop((c, j))
            rc = nrm.tile([1, 1024], f32, tag="rc", bufs=2)
            nc.vector.reciprocal_approx_fast(rc, lp)
            rcb = nrm.tile([1, 1024], bf16, tag="rcb", bufs=2)
            nc.vector.tensor_copy(rcb, rc)
            bc = nrm.tile([P, 1024], bf16, tag="bc", bufs=3)
            nc.gpsimd.partition_broadcast(bc[:, 0:512], rcb[0:1, 0:512])
            eng.tensor_tensor(
                yt_sb[0:D, c, jsl], yt_sb[0:D, c, jsl], bc[0:D, 0:512], MULT)
            nc.gpsimd.partition_broadcast(bc[:, 512:1024], rcb[0:1, 512:1024])
            if dbgB_d is not None:
                nc.sync.dma_start(dbgB_d[4 * j + c], rcb)
            nc.gpsimd.tensor_tensor(
                yt_sb[D:P, c, jsl], yt_sb[D:P, c, jsl],
                bc[D:P, 512:1024], MULT)

        # ------------------- emission schedule -------------------
        # j=0 starts as soon as quarter-0 V chains + pair-0 QK chains are
        # in; remaining projections and (later) out-proj row tiles are
        # popped as fillers between chunks, norm(c,j) one block after its
        # drain. All exp-pacing, PE never starved.
        for tti in range(4):
            piece_v(0, tti)
        # deferred x quarters 2-3: their triggers sit behind quarter-0's
        # affine_select in the gpsimd FIFO, so the transfers start only
        # once the head-critical DMAs are off the HBM bus
        nc.gpsimd.dma_start(x_sb[:, :, 1024:1536], xT_d[:, :, 1024:1536])
        nc.gpsimd.dma_start(x_sb[:, :, 1536:2048], xT_d[:, :, 1536:2048])
        piece_qk(0, 0)
        piece_qk(0, 4)

        F = []
        for ft in (1, 5, 2, 6, 3, 7):
            F.append(lambda ft=ft: piece_qk(0, ft))
        for tti in range(4):
            F.append(lambda tti=tti: piece_v(1, tti))

        att_block(0, 0, F, every=1)
        att_block(1, 0, F, every=1)
        att_block(2, 0, F, every=1)
        att_block(3, 0, F, every=1)
        while F:
            F.pop(0)()

        # j=1: fillers = quarter-2 V chains + T-half-1 QK chains (Q tiles
        # 2-3 + K quarters 2-3) + norms of tile 0
        F = [lambda: norm(0, 0)]
        for tti in range(4):
            F.append(lambda tti=tti: piece_v(2, tti))
        F.append(lambda: norm(1, 0))
        for ft in (0, 4, 1, 5):
            F.append(lambda ft=ft: piece_qk(1, ft))
        F.append(lambda: norm(2, 0))
        F.append(lambda: norm(3, 0))

        att_block(0, 1, F, every=2)
        att_block(1, 1, F, every=2)
        att_block(2, 1, F, every=2)
        att_block(3, 1, F, every=2)
        while F:
            F.pop(0)()

        # j=2: fillers = quarter-3 V chains + remaining QK + norms of tile 1
        F = [lambda: norm(0, 1)]
        for tti in range(4):
            F.append(lambda tti=tti: piece_v(3, tti))
        F.append(lambda: norm(1, 1))
        for ft in (2, 6, 3, 7):
            F.append(lambda ft=ft: piece_qk(1, ft))
        F.append(lambda: norm(2, 1))
        F.append(lambda: norm(3, 1))

        att_block(0, 2, F, every=2)
        att_block(1, 2, F, every=2)
        att_block(2, 2, F, every=2)
        att_block(3, 2, F, every=2)
        while F:
            F.pop(0)()

        # projections done: swap the proj PSUM banks for the out-proj pool
        psp_ctx.close()
        pools["pso"] = top.enter_context(
            tc.tile_pool(name="pso", bufs=2, space="PSUM"))

        # j=3: fillers = norms of tile 2 + out-proj of quarters 0-2
        F = [lambda: norm(0, 2)]
        F.append(lambda: piece_out(0))
        F.append(lambda: norm(1, 2))
        for tt in (1, 2):
            F.append(lambda tt=tt: piece_out(tt))
        F.append(lambda: norm(2, 2))
        for tt in (3, 4):
            F.append(lambda tt=tt: piece_out(tt))
        F.append(lambda: norm(3, 2))
        for tt in (5, 6, 7, 8):
            F.append(lambda tt=tt: piece_out(tt))

        att_block(0, 3, F, every=2)
        F.append(lambda: norm(0, 3))
        for tt in (9, 10):
            F.append(lambda tt=tt: piece_out(tt))
        att_block(1, 3, F, every=2)
        F.append(lambda: norm(1, 3))
        F.append(lambda: piece_out(11))
        att_block(2, 3, F, every=2)
        F.append(lambda: norm(2, 3))
        att_block(3, 3, F, every=2)
        while F:
            F.pop(0)()
        norm(3, 3)
        for tt in range(12, 16):
            piece_out(tt)



def _shard_inputs(x, w_qkv, w_out):
    mask = np.tril(np.ones((P, P), dtype=np.float32)).T
    mask = mask.astype(ml_dtypes.bfloat16)
    in_maps = []
    for core in range(8):
        b, hg = core // 2, core % 2
        sl = slice(hg * 512, (hg + 1) * 512)
        wq = w_qkv[0:1024][sl]
        wk = w_qkv[1024:2048][sl]
        wv = w_qkv[2048:3072][sl]
        wqkvT = np.concatenate([wq, wk, wv], axis=0).T  # [E, 1536]
        wvP = wqkvT[:, 1024:1536].reshape(NE, P, 512).transpose(1, 0, 2)
        wqkP = wqkvT[:, 0:1024].reshape(NE, P, 8, P).transpose(1, 2, 0, 3)
        # interleave the ft axis in attention need-order: 0,4,1,5,2,6,3,7
        wqkP = wqkP[:, (0, 4, 1, 5, 2, 6, 3, 7)]
        woT = w_out[:, sl].T  # [512, E]
        woP = woT.reshape(4, P, E).transpose(1, 0, 2)
        xT = x[b].T.reshape(NE, P, T).transpose(1, 0, 2)  # [P, NE, T]
        cvt = lambda a: np.ascontiguousarray(a).astype(ml_dtypes.bfloat16)
        in_maps.append({
            "xT": cvt(xT),
            "wvP": cvt(wvP),
            "wqkP": cvt(wqkP),
            "woP": cvt(woP),
            "mask": mask,
        })
    return in_maps


def kernel(x, w_qkv, w_out, _trace=False):
    x = np.asarray(x, dtype=np.float32)
    w_qkv = np.asarray(w_qkv, dtype=np.float32)
    w_out = np.asarray(w_out, dtype=np.float32)

    if "nc" not in _CACHE:
        _CACHE["nc"] = build()
    nc = _CACHE["nc"]

    in_maps = _shard_inputs(x, w_qkv, w_out)
    res = bass_utils.run_bass_kernel_spmd(
        nc, in_maps, core_ids=list(range(8)), trace=_trace)
    kernel.last_result = res

    out = np.empty((B, T, E), dtype=np.float32)
    for b in range(B):
        out[b] = (res.results[2 * b]["out"].astype(np.float32)
                  + res.results[2 * b + 1]["out"].astype(np.float32))
    return out


# revision 44
# speedup vs baseline: 1.3353x; 1.3353x over previous
"""Causal self-attention Trainium2 kernel (B=4, T=2048, E=1024, H=16, D=64).

Sharding: 8 cores = batch(4) x head-group(2). Each core computes the full
attention for 8 heads of one batch element plus its half of the output
projection; the host sums the two out-proj partials per batch element.

v3 dataflow (per core, all matmul operands bf16, PSUM f32):
  - x^T [E,T] lives fully in SBUF, loaded with 4 large DMAs (one per
    T-quarter) on two queues so the first V-proj chain starts ~4us in.
  - Projection chains (V then QK per quarter) are interleaved into the
    attention stream as fillers so the PE never idles while ScalarE works
    through the exps; out-projection row tiles become fillers as soon as
    their query quarter is normalized.
  - S^T chunks [128kv, 2x512q]: two heads of a pair issued as row-group
    tiled matmul pairs (tile_position (0,0)/(64,0)) running concurrently.
    Diagonal chunks only compute live columns; exp on ScalarE -> bf16;
    causal mask via DVE multiply with a [128,640] zeros|tril constant.
  - y^T accumulation [65,512] per head, lhsT = V_aug (ones column carries
    the softmax denominator through the PV matmul).
  - Drain per (pair, tile): DVE casts y rows to yt/tmpb, tmpb is DMA'd
    into yt's upper partitions immediately (not norm-gated), DVE
    reciprocal runs directly on the PSUM denominator rows (partition 64),
    one cast + one tiny DMA lands both rcp rows on a partition-0 table.
  - Norm per (pair, tile), emitted ~one block later so every op's deps are
    met when it reaches its engine FIFO: 2 gpsimd partition_broadcasts +
    2 in-place DVE multiplies on yt. No cross-engine convoys.
  - ScalarE exp table is pre-warmed during the prologue DMAs.
"""

import numpy as np
import ml_dtypes

import concourse.bass as bass
import concourse.bacc as bacc
import concourse.mybir as mybir
import concourse.tile as tile
from concourse import bass_utils

f32 = mybir.dt.float32
bf16 = mybir.dt.bfloat16
FP = mybir.dt.float32  # psum dtype

P = 128
B, T, E = 4, 2048, 1024
H, D = 16, 64
HPC = H // 2            # heads per core = 8
NE = E // P             # 8 e-chunks
NTT = T // P            # 16 kv chunks
NQ = T // 512           # 4 query tiles of 512
SCALE = 1.0 / np.sqrt(D)

Exp = mybir.ActivationFunctionType.Exp
MULT = mybir.AluOpType.mult
IS_GE = mybir.AluOpType.is_ge

_CACHE = {}


def build(**opts):
    nc = bacc.Bacc("TRN2", target_bir_lowering=False, debug=False, num_devices=8)

    xT_d = nc.dram_tensor("xT", [P, NE, T], bf16, kind="ExternalInput")
    wvP_d = nc.dram_tensor("wvP", [P, NE, 512], bf16, kind="ExternalInput")
    wqkP_d = nc.dram_tensor("wqkP", [P, 8, NE, P], bf16, kind="ExternalInput")
    woP_d = nc.dram_tensor("woP", [P, 4, E], bf16, kind="ExternalInput")
    mask_d = nc.dram_tensor("mask", [P, P], bf16, kind="ExternalInput")
    out_d = nc.dram_tensor("out", [T, E], bf16, kind="ExternalOutput")
    dbgL_d = dbgB_d = None
    if opts.pop("debug_rcp", False):
        dbgL_d = nc.dram_tensor("dbgL", [16, 1024], f32,
                                kind="ExternalOutput")
        dbgB_d = nc.dram_tensor("dbgB", [16, 1024], bf16,
                                kind="ExternalOutput")

    with tile.TileContext(nc) as tc:
        build_body(tc, xT_d, wvP_d, wqkP_d, woP_d, mask_d, out_d,
                   dbgL_d=dbgL_d, dbgB_d=dbgB_d, **opts)
    nc.compile()
    return nc


def build_body(tc, xT_d, wvP_d, wqkP_d, woP_d, mask_d, out_d,
               pss_bufs=2, psy_bufs=2, norm_mode="full",
               dbgL_d=None, dbgB_d=None):
    nc = tc.nc

    from contextlib import ExitStack
    with ExitStack() as top:
        per = top.enter_context(tc.tile_pool(name="per", bufs=1))

        qk_sb = per.tile([P, 8, T], bf16)            # chunks 0-3: Q^T, 4-7: K^T
        v_sb = per.tile([P, NTT, HPC, D + 1], bf16)  # [kv_p, kv_chunk, head, d|1]
        yt_sb = per.tile([P, 4, T], bf16)            # [f%128, f//128, q]
        x_sb = per.tile([P, NE, T], bf16)            # x^T resident [p, e, t]
        wv_sb = per.tile([P, NE, 512], bf16)         # V-proj weights
        wqk_sb = per.tile([P, 8, NE, P], bf16)       # QK-proj weights per f-chunk
        wo_sb = per.tile([P, 4, E], bf16)            # out-proj weights
        mask_sb = per.tile([P, P], bf16)             # tril(128).T
        warm_sb = per.tile([4, 512], bf16)           # gpsimd ucode warmup dst
        # self-managed ptt ring: stale regions are never read (exp writes
        # [q0:512] per head and PV streams only those columns)
        ptt_ring = [per.tile([P, 1024], bf16, name=f"ptt{k}")
                    for k in range(4)]
        ptt_ctr = [0]

        # --------- prologue DMAs: 4 big x loads + weights, spread across
        # queues; warm the gpsimd ucode library and the ScalarE exp table
        # during the transfer ----------
        nc.gpsimd.partition_broadcast(warm_sb, ptt_ring[1][0:1, 0:512])
        # Priority-ordered prologue: the head is HBM-bound on 7MB of
        # input, but the first V chain needs only x-q0 + wv (2MB). Those
        # go first on the two HW queues; wqk pairs follow in the order
        # the attention needs them (host pre-interleaves ft as
        # 0,4,1,5,2,6,3,7); x quarters 2-3 are triggered later (their
        # dma_starts are emitted after quarter-0 V finishes, see below).
        nc.sync.dma_start(x_sb[:, :, 0:512], xT_d[:, :, 0:512])
        nc.scalar.dma_start(wv_sb, wvP_d[:, :, :])
        nc.sync.dma_start(wqk_sb[:, 0:2], wqkP_d[:, 0:2])
        nc.scalar.dma_start(wqk_sb[:, 2:4], wqkP_d[:, 2:4])
        nc.sync.dma_start(x_sb[:, :, 512:1024], xT_d[:, :, 512:1024])
        nc.scalar.dma_start(wqk_sb[:, 4:6], wqkP_d[:, 4:6])
        nc.scalar.dma_start(wqk_sb[:, 6:8], wqkP_d[:, 6:8])
        nc.scalar.dma_start(wo_sb, woP_d[:, :, :])
        nc.sync.dma_start(mask_sb, mask_d[:, :])
        # warm the exp table-set while DMAs fly (first ACT pays ~2.7us)
        nc.scalar.activation(warm_sb[:, 0:8], warm_sb[:, 0:8],
                             Exp, scale=1.0)
        # logical ft -> position in the host-interleaved wqk layout
        ftpos = {0: 0, 4: 1, 1: 2, 5: 3, 2: 4, 6: 5, 3: 6, 7: 7}

        # pool creation order: psp LAST so it sits on top of the PSUM stack
        # and can be swapped for the out-proj pool after projections end
        drn = top.enter_context(tc.tile_pool(name="drn", bufs=2))
        nrm = top.enter_context(tc.tile_pool(name="nrm", bufs=3))
        ost = top.enter_context(tc.tile_pool(name="ost", bufs=2))
        pss = top.enter_context(
            tc.tile_pool(name="pss", bufs=pss_bufs, space="PSUM"))
        psy = top.enter_context(
            tc.tile_pool(name="psy", bufs=psy_bufs, space="PSUM"))
        psp_ctx = ExitStack()
        psp = psp_ctx.enter_context(
            tc.tile_pool(name="psp", bufs=1, space="PSUM"))
        pools = {}
        done = set()
        lps = {}

        def piece_v(th, tti):
            # one V-projection chain: v_sb chunk tt, natural layout
            if ("v", th, tti) in done:
                return
            done.add(("v", th, tti))
            tt = th * 4 + tti
            ps = psp.tile([P, 1024], FP, tag="pq")
            for e in range(NE):
                nc.tensor.matmul(
                    ps[:, 0:512],
                    lhsT=x_sb[:, e, tt * P:(tt + 1) * P],
                    rhs=wv_sb[:, e, :],
                    start=(e == 0), stop=(e == NE - 1))
            nc.vector.tensor_copy(
                v_sb[:, tt, :, 0:D],
                ps[:, 0:512].rearrange("p (h d) -> p h d", h=HPC))
            if tti == 3:
                # ones column for this quarter (never keeps in_: cond<0)
                ov = v_sb[:, th * 4:(th + 1) * 4, :, D:D + 1]
                iv = v_sb[:, th * 4:(th + 1) * 4, :, 0:1]
                nc.gpsimd.affine_select(
                    ov, iv, pattern=[[0, 4], [0, HPC], [0, 1]],
                    compare_op=IS_GE, fill=1.0, base=-1,
                    channel_multiplier=0)

        def piece_qk(hf, ft):
            # one QK-projection chain over a T-half: each weight load
            # feeds two N=512 matmuls (adjacent quarters, same lhsT)
            if ("qk", hf, ft) in done:
                return
            done.add(("qk", hf, ft))
            h0 = hf * 1024
            ps = psp.tile([P, 1024], FP, tag="pq")
            for e in range(NE):
                nc.tensor.matmul(
                    ps[:, 0:512],
                    lhsT=wqk_sb[:, ftpos[ft], e, :],
                    rhs=x_sb[:, e, h0:h0 + 512],
                    start=(e == 0), stop=(e == NE - 1))
                nc.tensor.matmul(
                    ps[:, 512:1024],
                    lhsT=wqk_sb[:, ftpos[ft], e, :],
                    rhs=x_sb[:, e, h0 + 512:h0 + 1024],
                    start=(e == 0), stop=(e == NE - 1))
            nc.vector.tensor_copy(
                qk_sb[:, ft, h0:h0 + 1024], ps)

        def piece_out(tt):
            # one out-proj row tile, both halves: each yt weight load
            # feeds two N=512 matmuls into the two po PSUM buffers
            pso = pools["pso"]
            po0 = pso.tile([P, 512], FP, tag="po", name=f"po0_{tt}")
            po1 = pso.tile([P, 512], FP, tag="po", name=f"po1_{tt}")
            for c2 in range(4):
                nc.tensor.matmul(
                    po0,
                    lhsT=yt_sb[:, c2, tt * P:(tt + 1) * P],
                    rhs=wo_sb[:, c2, 0:512],
                    start=(c2 == 0), stop=(c2 == 3))
                nc.tensor.matmul(
                    po1,
                    lhsT=yt_sb[:, c2, tt * P:(tt + 1) * P],
                    rhs=wo_sb[:, c2, 512:1024],
                    start=(c2 == 0), stop=(c2 == 3))
            st = ost.tile([P, 1024], bf16, tag="st")
            nc.vector.tensor_copy(st[:, 0:512], po0)
            nc.vector.tensor_copy(st[:, 512:1024], po1)
            nc.sync.dma_start(out_d[tt * P:(tt + 1) * P, :], st)

        def att_block(c, j, fillers=None, every=4):
            """Attention for head pair (2c, 2c+1), query tile j.

            Pops one filler (independent work) every `every` chunks so the
            PE stays busy while ScalarE works through the exps."""
            jsl = slice(j * 512, (j + 1) * 512)
            nkv = 4 * j + 4
            yps = [psy.tile([D + 1, 512], FP, tag="y", name=f"yps_{c}_{j}_{k}")
                   for k in range(2)]
            for i in range(nkv):
                off = i - 4 * j
                q0 = max(0, 128 * off)
                spt = pss.tile([P, 1024], FP, tag="s")
                for hh in range(2):
                    p0 = 64 * hh
                    nc.tensor.matmul(
                        spt[:, hh * 512 + q0:(hh + 1) * 512],
                        lhsT=qk_sb[p0:p0 + 64, 4 + c, i * P:(i + 1) * P],
                        rhs=qk_sb[p0:p0 + 64, c, j * 512 + q0:(j + 1) * 512],
                        start=True, stop=True,
                        tile_position=(p0, 0))
                ptt = ptt_ring[ptt_ctr[0] % 4]
                ptt_ctr[0] += 1
                if off < 0:
                    nc.scalar.activation(ptt, spt, Exp, scale=float(SCALE))
                else:
                    pv = ptt.rearrange("p (h q) -> p h q", h=2)
                    sv = spt.rearrange("p (h q) -> p h q", h=2)
                    nc.scalar.activation(pv[:, :, q0:512], sv[:, :, q0:512],
                                         Exp, scale=float(SCALE))
                    # causal mask: zero the dead upper triangle of the
                    # [q0,q0+128) block; columns below q0 are never read
                    for hx in range(2):
                        nc.vector.tensor_tensor(
                            pv[:, hx, q0:q0 + P], pv[:, hx, q0:q0 + P],
                            mask_sb, MULT)
                for hh in range(2):
                    # stream only the live columns [q0:512]; dead columns
                    # keep earlier (sub-diagonal) partial sums, which is
                    # exact since dead chunks contribute zero there
                    nc.tensor.matmul(
                        yps[hh][:, q0:512],
                        lhsT=v_sb[:, i, 2 * c + hh, :],
                        rhs=ptt[:, hh * 512 + q0:(hh + 1) * 512],
                        start=(i == 0), stop=(i == nkv - 1),
                        skip_group_check=True)
                if fillers and (i + 1) % every == 0 and fillers:
                    fillers.pop(0)()
            # drain: denominator path first (it gates the norm chain),
            # then the y-row casts. Rows are staged to partition 64 of
            # stg and DMA-shifted to a partition-0 tile for the norm
            # (custom-DVE/gpsimd ops mishandle nonzero base partitions).
            stg = drn.tile([D + 1, 1024], f32, tag="stg")
            nc.vector.tensor_copy(stg[D:D + 1, 0:512], yps[0][D:D + 1, :])
            nc.vector.tensor_copy(stg[D:D + 1, 512:1024], yps[1][D:D + 1, :])
            lp = nrm.tile([1, 1024], f32, tag="lp", bufs=4,
                          name=f"lp_{c}_{j}")
            nc.gpsimd.dma_start(lp, stg[D:D + 1, :])
            lps[(c, j)] = lp
            nc.vector.tensor_copy(yt_sb[0:D, c, jsl], yps[0][0:D, :])
            tmpb = drn.tile([D, 512], bf16, tag="tmpb", bufs=3)
            nc.vector.tensor_copy(tmpb, yps[1][0:D, :])
            nc.gpsimd.dma_start(yt_sb[64:128, c, jsl], tmpb)
            if dbgL_d is not None:
                nc.sync.dma_start(dbgL_d[4 * j + c], stg[D:D + 1, :])

        def norm(c, j, eng=None):
            """Normalize yt tile (c, j): two partition_broadcasts of the
            rcp rows + two in-place multiplies. Emitted ~a block after the
            drain so every dep is satisfied on arrival."""
            if norm_mode == "skip":
                return
            eng = eng or nc.vector
            jsl = slice(j * 512, (j + 1) * 512)
            lp = lps.pop((c, j))
            rc = nrm.tile([1, 1024], f32, tag="rc", bufs=2)
            nc.vector.reciprocal_approx_fast(rc, lp)
            rcb = nrm.tile([1, 1024], bf16, tag="rcb", bufs=2)
            nc.vector.tensor_copy(rcb, rc)
            bc = nrm.tile([P, 1024], bf16, tag="bc", bufs=3)
            nc.gpsimd.partition_broadcast(bc[:, 0:512], rcb[0:1, 0:512])
            eng.tensor_tensor(
                yt_sb[0:D, c, jsl], yt_sb[0:D, c, jsl], bc[0:D, 0:512], MULT)
            nc.gpsimd.partition_broadcast(bc[:, 512:1024], rcb[0:1, 512:1024])
            if dbgB_d is not None:
                nc.sync.dma_start(dbgB_d[4 * j + c], rcb)
            eng.tensor_tensor(
                yt_sb[D:P, c, jsl], yt_sb[D:P, c, jsl],
                bc[D:P, 512:1024], MULT)

        # ------------------- emission schedule -------------------
        # j=0 starts as soon as quarter-0 V chains + pair-0 QK chains are
        # in; remaining projections and (later) out-proj row tiles are
        # popped as fillers between chunks, norm(c,j) one block after its
        # drain. All exp-pacing, PE never starved.
        for tti in range(4):
            piece_v(0, tti)
        # deferred x quarters 2-3: their triggers sit behind quarter-0's
        # affine_select in the gpsimd FIFO, so the transfers start only
        # once the head-critical DMAs are off the HBM bus
        nc.gpsimd.dma_start(x_sb[:, :, 1024:1536], xT_d[:, :, 1024:1536])
        nc.gpsimd.dma_start(x_sb[:, :, 1536:2048], xT_d[:, :, 1536:2048])
        piece_qk(0, 0)
        piece_qk(0, 4)

        F = []
        for ft in (1, 5, 2, 6, 3, 7):
            F.append(lambda ft=ft: piece_qk(0, ft))
        for tti in range(4):
            F.append(lambda tti=tti: piece_v(1, tti))

        att_block(0, 0, F, every=1)
        att_block(1, 0, F, every=1)
        att_block(2, 0, F, every=1)
        att_block(3, 0, F, every=1)
        while F:
            F.pop(0)()

        # j=1: fillers = quarter-2 V chains + T-half-1 QK chains (Q tiles
        # 2-3 + K quarters 2-3) + norms of tile 0
        F = [lambda: norm(0, 0)]
        for tti in range(4):
            F.append(lambda tti=tti: piece_v(2, tti))
        F.append(lambda: norm(1, 0))
        for ft in (0, 4, 1, 5):
            F.append(lambda ft=ft: piece_qk(1, ft))
        F.append(lambda: norm(2, 0))
        F.append(lambda: norm(3, 0))

        att_block(0, 1, F, every=2)
        att_block(1, 1, F, every=2)
        att_block(2, 1, F, every=2)
        att_block(3, 1, F, every=2)
        while F:
            F.pop(0)()

        # j=2: fillers = quarter-3 V chains + remaining QK + norms of tile 1
        F = [lambda: norm(0, 1)]
        for tti in range(4):
            F.append(lambda tti=tti: piece_v(3, tti))
        F.append(lambda: norm(1, 1))
        for ft in (2, 6, 3, 7):
            F.append(lambda ft=ft: piece_qk(1, ft))
        F.append(lambda: norm(2, 1))
        F.append(lambda: norm(3, 1))

        att_block(0, 2, F, every=2)
        att_block(1, 2, F, every=2)
        att_block(2, 2, F, every=2)
        att_block(3, 2, F, every=2)
        while F:
            F.pop(0)()

        # projections done: swap the proj PSUM banks for the out-proj pool
        psp_ctx.close()
        pools["pso"] = top.enter_context(
            tc.tile_pool(name="pso", bufs=2, space="PSUM"))

        # j=3: fillers = norms of tile 2 + out-proj of quarters 0-2
        F = [lambda: norm(0, 2)]
        F.append(lambda: piece_out(0))
        F.append(lambda: norm(1, 2))
        for tt in (1, 2):
            F.append(lambda tt=tt: piece_out(tt))
        F.append(lambda: norm(2, 2))
        for tt in (3, 4):
            F.append(lambda tt=tt: piece_out(tt))
        F.append(lambda: norm(3, 2))
        for tt in (5, 6, 7, 8):
            F.append(lambda tt=tt: piece_out(tt))

        att_block(0, 3, F, every=2)
        F.append(lambda: norm(0, 3))
        for tt in (9, 10):
            F.append(lambda tt=tt: piece_out(tt))
        att_block(1, 3, F, every=2)
        F.append(lambda: norm(1, 3))
        F.append(lambda: piece_out(11))
        att_block(2, 3, F, every=2)
        F.append(lambda: norm(2, 3))
        att_block(3, 3, F, every=2)
        while F:
            F.pop(0)()
        norm(3, 3)
        for tt in range(12, 16):
            piece_out(tt)



def _shard_inputs(x, w_qkv, w_out):
    mask = np.tril(np.ones((P, P), dtype=np.float32)).T
    mask = mask.astype(ml_dtypes.bfloat16)
    in_maps = []
    for core in range(8):
        b, hg = core // 2, core % 2
        sl = slice(hg * 512, (hg + 1) * 512)
        wq = w_qkv[0:1024][sl]
        wk = w_qkv[1024:2048][sl]
        wv = w_qkv[2048:3072][sl]
        wqkvT = np.concatenate([wq, wk, wv], axis=0).T  # [E, 1536]
        wvP = wqkvT[:, 1024:1536].reshape(NE, P, 512).transpose(1, 0, 2)
        wqkP = wqkvT[:, 0:1024].reshape(NE, P, 8, P).transpose(1, 2, 0, 3)
        # interleave the ft axis in attention need-order: 0,4,1,5,2,6,3,7
        wqkP = wqkP[:, (0, 4, 1, 5, 2, 6, 3, 7)]
        woT = w_out[:, sl].T  # [512, E]
        woP = woT.reshape(4, P, E).transpose(1, 0, 2)
        xT = x[b].T.reshape(NE, P, T).transpose(1, 0, 2)  # [P, NE, T]
        cvt = lambda a: np.ascontiguousarray(a).astype(ml_dtypes.bfloat16)
        in_maps.append({
            "xT": cvt(xT),
            "wvP": cvt(wvP),
            "wqkP": cvt(wqkP),
            "woP": cvt(woP),
            "mask": mask,
        })
    return in_maps


def kernel(x, w_qkv, w_out, _trace=False):
    x = np.asarray(x, dtype=np.float32)
    w_qkv = np.asarray(w_qkv, dtype=np.float32)
    w_out = np.asarray(w_out, dtype=np.float32)

    if "nc" not in _CACHE:
        _CACHE["nc"] = build()
    nc = _CACHE["nc"]

    in_maps = _shard_inputs(x, w_qkv, w_out)
    res = bass_utils.run_bass_kernel_spmd(
        nc, in_maps, core_ids=list(range(8)), trace=_trace)
    kernel.last_result = res

    out = np.empty((B, T, E), dtype=np.float32)
    for b in range(B):
        out[b] = (res.results[2 * b]["out"].astype(np.float32)
                  + res.results[2 * b + 1]["out"].astype(np.float32))
    return out


# revision 45
# speedup vs baseline: 1.3686x; 1.0249x over previous
"""Causal self-attention Trainium2 kernel (B=4, T=2048, E=1024, H=16, D=64).

Sharding: 8 cores = batch(4) x head-group(2). Each core computes the full
attention for 8 heads of one batch element plus its half of the output
projection; the host sums the two out-proj partials per batch element.

v3 dataflow (per core, all matmul operands bf16, PSUM f32):
  - x^T [E,T] lives fully in SBUF, loaded with 4 large DMAs (one per
    T-quarter) on two queues so the first V-proj chain starts ~4us in.
  - Projection chains (V then QK per quarter) are interleaved into the
    attention stream as fillers so the PE never idles while ScalarE works
    through the exps; out-projection row tiles become fillers as soon as
    their query quarter is normalized.
  - S^T chunks [128kv, 2x512q]: two heads of a pair issued as row-group
    tiled matmul pairs (tile_position (0,0)/(64,0)) running concurrently.
    Diagonal chunks only compute live columns; exp on ScalarE -> bf16;
    causal mask via DVE multiply with a [128,640] zeros|tril constant.
  - y^T accumulation [65,512] per head, lhsT = V_aug (ones column carries
    the softmax denominator through the PV matmul).
  - Drain per (pair, tile): DVE casts y rows to yt/tmpb, tmpb is DMA'd
    into yt's upper partitions immediately (not norm-gated), DVE
    reciprocal runs directly on the PSUM denominator rows (partition 64),
    one cast + one tiny DMA lands both rcp rows on a partition-0 table.
  - Norm per (pair, tile), emitted ~one block later so every op's deps are
    met when it reaches its engine FIFO: 2 gpsimd partition_broadcasts +
    2 in-place DVE multiplies on yt. No cross-engine convoys.
  - ScalarE exp table is pre-warmed during the prologue DMAs.
"""

import numpy as np
import ml_dtypes

import concourse.bass as bass
import concourse.bacc as bacc
import concourse.mybir as mybir
import concourse.tile as tile
from concourse import bass_utils

f32 = mybir.dt.float32
bf16 = mybir.dt.bfloat16
FP = mybir.dt.float32  # psum dtype

P = 128
B, T, E = 4, 2048, 1024
H, D = 16, 64
HPC = H // 2            # heads per core = 8
NE = E // P             # 8 e-chunks
NTT = T // P            # 16 kv chunks
NQ = T // 512           # 4 query tiles of 512
SCALE = 1.0 / np.sqrt(D)

Exp = mybir.ActivationFunctionType.Exp
MULT = mybir.AluOpType.mult
IS_GE = mybir.AluOpType.is_ge

_CACHE = {}


def build(**opts):
    nc = bacc.Bacc("TRN2", target_bir_lowering=False, debug=False, num_devices=8)

    xT_d = nc.dram_tensor("xT", [P, 4, NE, 512], bf16, kind="ExternalInput")
    wvP_d = nc.dram_tensor("wvP", [P, NE, 512], bf16, kind="ExternalInput")
    wqkP_d = nc.dram_tensor("wqkP", [P, 8, NE, P], bf16, kind="ExternalInput")
    woP_d = nc.dram_tensor("woP", [P, 4, E], bf16, kind="ExternalInput")
    mask_d = nc.dram_tensor("mask", [P, P], bf16, kind="ExternalInput")
    out_d = nc.dram_tensor("out", [T, E], bf16, kind="ExternalOutput")
    dbgL_d = dbgB_d = None
    if opts.pop("debug_rcp", False):
        dbgL_d = nc.dram_tensor("dbgL", [16, 1024], f32,
                                kind="ExternalOutput")
        dbgB_d = nc.dram_tensor("dbgB", [16, 1024], bf16,
                                kind="ExternalOutput")

    with tile.TileContext(nc) as tc:
        build_body(tc, xT_d, wvP_d, wqkP_d, woP_d, mask_d, out_d,
                   dbgL_d=dbgL_d, dbgB_d=dbgB_d, **opts)
    nc.compile()
    return nc


def build_body(tc, xT_d, wvP_d, wqkP_d, woP_d, mask_d, out_d,
               pss_bufs=2, psy_bufs=2, norm_mode="full",
               dbgL_d=None, dbgB_d=None):
    nc = tc.nc

    from contextlib import ExitStack
    with ExitStack() as top:
        per = top.enter_context(tc.tile_pool(name="per", bufs=1))

        qk_sb = per.tile([P, 8, T], bf16)            # chunks 0-3: Q^T, 4-7: K^T
        v_sb = per.tile([P, NTT, HPC, D + 1], bf16)  # [kv_p, kv_chunk, head, d|1]
        yt_sb = per.tile([P, 4, T], bf16)            # [f%128, f//128, q]
        x_sb = per.tile([P, 4, NE, 512], bf16)       # x^T resident [p, th, e, t512]
        wv_sb = per.tile([P, NE, 512], bf16)         # V-proj weights
        wqk_sb = per.tile([P, 8, NE, P], bf16)       # QK-proj weights per f-chunk
        wo_sb = per.tile([P, 4, E], bf16)            # out-proj weights
        mask_sb = per.tile([P, P], bf16)             # tril(128).T
        warm_sb = per.tile([4, 512], bf16)           # gpsimd ucode warmup dst
        # self-managed ptt ring: stale regions are never read (exp writes
        # [q0:512] per head and PV streams only those columns)
        ptt_ring = [per.tile([P, 1024], bf16, name=f"ptt{k}")
                    for k in range(4)]
        ptt_ctr = [0]

        # --------- prologue DMAs: 4 big x loads + weights, spread across
        # queues; warm the gpsimd ucode library and the ScalarE exp table
        # during the transfer ----------
        nc.gpsimd.partition_broadcast(warm_sb, ptt_ring[1][0:1, 0:512])
        # Priority-ordered prologue: the head is HBM-bound on 7MB of
        # input, but the first V chain needs only x-q0 + wv (2MB). Those
        # go first on the two HW queues; wqk pairs follow in the order
        # the attention needs them (host pre-interleaves ft as
        # 0,4,1,5,2,6,3,7); x quarters 2-3 are triggered later (their
        # dma_starts are emitted after quarter-0 V finishes, see below).
        nc.sync.dma_start(x_sb[:, 0], xT_d[:, 0])
        nc.scalar.dma_start(wv_sb, wvP_d[:, :, :])
        nc.sync.dma_start(wqk_sb[:, 0:2], wqkP_d[:, 0:2])
        nc.scalar.dma_start(wqk_sb[:, 2:4], wqkP_d[:, 2:4])
        nc.sync.dma_start(x_sb[:, 1], xT_d[:, 1])
        nc.scalar.dma_start(wqk_sb[:, 4:6], wqkP_d[:, 4:6])
        nc.scalar.dma_start(wqk_sb[:, 6:8], wqkP_d[:, 6:8])
        nc.scalar.dma_start(wo_sb, woP_d[:, :, :])
        nc.sync.dma_start(mask_sb, mask_d[:, :])
        # warm the exp table-set while DMAs fly (first ACT pays ~2.7us)
        nc.scalar.activation(warm_sb[:, 0:8], warm_sb[:, 0:8],
                             Exp, scale=1.0)
        # logical ft -> position in the host-interleaved wqk layout
        ftpos = {0: 0, 4: 1, 1: 2, 5: 3, 2: 4, 6: 5, 3: 6, 7: 7}

        # pool creation order: psp LAST so it sits on top of the PSUM stack
        # and can be swapped for the out-proj pool after projections end
        drn = top.enter_context(tc.tile_pool(name="drn", bufs=2))
        nrm = top.enter_context(tc.tile_pool(name="nrm", bufs=3))
        ost = top.enter_context(tc.tile_pool(name="ost", bufs=2))
        pss = top.enter_context(
            tc.tile_pool(name="pss", bufs=pss_bufs, space="PSUM"))
        psy = top.enter_context(
            tc.tile_pool(name="psy", bufs=psy_bufs, space="PSUM"))
        psp_ctx = ExitStack()
        psp = psp_ctx.enter_context(
            tc.tile_pool(name="psp", bufs=1, space="PSUM"))
        pools = {}
        done = set()
        lps = {}

        def piece_v(th, tti):
            # one V-projection chain: v_sb chunk tt, natural layout
            if ("v", th, tti) in done:
                return
            done.add(("v", th, tti))
            tt = th * 4 + tti
            ps = psp.tile([P, 1024], FP, tag="pq")
            for e in range(NE):
                nc.tensor.matmul(
                    ps[:, 0:512],
                    lhsT=x_sb[:, th, e, tti * P:(tti + 1) * P],
                    rhs=wv_sb[:, e, :],
                    start=(e == 0), stop=(e == NE - 1))
            nc.vector.tensor_copy(
                v_sb[:, tt, :, 0:D],
                ps[:, 0:512].rearrange("p (h d) -> p h d", h=HPC))
            if tti == 3:
                # ones column for this quarter (never keeps in_: cond<0)
                ov = v_sb[:, th * 4:(th + 1) * 4, :, D:D + 1]
                iv = v_sb[:, th * 4:(th + 1) * 4, :, 0:1]
                nc.gpsimd.affine_select(
                    ov, iv, pattern=[[0, 4], [0, HPC], [0, 1]],
                    compare_op=IS_GE, fill=1.0, base=-1,
                    channel_multiplier=0)

        def piece_qk(hf, ft):
            # one QK-projection chain over a T-half: each weight load
            # feeds two N=512 matmuls (adjacent quarters, same lhsT)
            if ("qk", hf, ft) in done:
                return
            done.add(("qk", hf, ft))
            h0 = hf * 1024
            ps = psp.tile([P, 1024], FP, tag="pq")
            for e in range(NE):
                nc.tensor.matmul(
                    ps[:, 0:512],
                    lhsT=wqk_sb[:, ftpos[ft], e, :],
                    rhs=x_sb[:, 2 * hf, e, :],
                    start=(e == 0), stop=(e == NE - 1))
                nc.tensor.matmul(
                    ps[:, 512:1024],
                    lhsT=wqk_sb[:, ftpos[ft], e, :],
                    rhs=x_sb[:, 2 * hf + 1, e, :],
                    start=(e == 0), stop=(e == NE - 1))
            nc.vector.tensor_copy(
                qk_sb[:, ft, h0:h0 + 1024], ps)

        def piece_out(tt):
            # one out-proj row tile, both halves: each yt weight load
            # feeds two N=512 matmuls into the two po PSUM buffers
            pso = pools["pso"]
            po0 = pso.tile([P, 512], FP, tag="po", name=f"po0_{tt}")
            po1 = pso.tile([P, 512], FP, tag="po", name=f"po1_{tt}")
            for c2 in range(4):
                nc.tensor.matmul(
                    po0,
                    lhsT=yt_sb[:, c2, tt * P:(tt + 1) * P],
                    rhs=wo_sb[:, c2, 0:512],
                    start=(c2 == 0), stop=(c2 == 3))
                nc.tensor.matmul(
                    po1,
                    lhsT=yt_sb[:, c2, tt * P:(tt + 1) * P],
                    rhs=wo_sb[:, c2, 512:1024],
                    start=(c2 == 0), stop=(c2 == 3))
            st = ost.tile([P, 1024], bf16, tag="st")
            nc.vector.tensor_copy(st[:, 0:512], po0)
            nc.vector.tensor_copy(st[:, 512:1024], po1)
            nc.sync.dma_start(out_d[tt * P:(tt + 1) * P, :], st)

        def att_block(c, j, fillers=None, every=4):
            """Attention for head pair (2c, 2c+1), query tile j.

            Pops one filler (independent work) every `every` chunks so the
            PE stays busy while ScalarE works through the exps."""
            jsl = slice(j * 512, (j + 1) * 512)
            nkv = 4 * j + 4
            yps = [psy.tile([D + 1, 512], FP, tag="y", name=f"yps_{c}_{j}_{k}")
                   for k in range(2)]
            for i in range(nkv):
                off = i - 4 * j
                q0 = max(0, 128 * off)
                spt = pss.tile([P, 1024], FP, tag="s")
                for hh in range(2):
                    p0 = 64 * hh
                    nc.tensor.matmul(
                        spt[:, hh * 512 + q0:(hh + 1) * 512],
                        lhsT=qk_sb[p0:p0 + 64, 4 + c, i * P:(i + 1) * P],
                        rhs=qk_sb[p0:p0 + 64, c, j * 512 + q0:(j + 1) * 512],
                        start=True, stop=True,
                        tile_position=(p0, 0))
                ptt = ptt_ring[ptt_ctr[0] % 4]
                ptt_ctr[0] += 1
                if off < 0:
                    nc.scalar.activation(ptt, spt, Exp, scale=float(SCALE))
                else:
                    pv = ptt.rearrange("p (h q) -> p h q", h=2)
                    sv = spt.rearrange("p (h q) -> p h q", h=2)
                    nc.scalar.activation(pv[:, :, q0:512], sv[:, :, q0:512],
                                         Exp, scale=float(SCALE))
                    # causal mask: zero the dead upper triangle of the
                    # [q0,q0+128) block; columns below q0 are never read
                    for hx in range(2):
                        nc.vector.tensor_tensor(
                            pv[:, hx, q0:q0 + P], pv[:, hx, q0:q0 + P],
                            mask_sb, MULT)
                for hh in range(2):
                    # stream only the live columns [q0:512]; dead columns
                    # keep earlier (sub-diagonal) partial sums, which is
                    # exact since dead chunks contribute zero there
                    nc.tensor.matmul(
                        yps[hh][:, q0:512],
                        lhsT=v_sb[:, i, 2 * c + hh, :],
                        rhs=ptt[:, hh * 512 + q0:(hh + 1) * 512],
                        start=(i == 0), stop=(i == nkv - 1),
                        skip_group_check=True)
                if fillers and (i + 1) % every == 0 and fillers:
                    fillers.pop(0)()
            # drain: denominator path first (it gates the norm chain),
            # then the y-row casts. Rows are staged to partition 64 of
            # stg and DMA-shifted to a partition-0 tile for the norm
            # (custom-DVE/gpsimd ops mishandle nonzero base partitions).
            stg = drn.tile([D + 1, 1024], f32, tag="stg")
            nc.vector.tensor_copy(stg[D:D + 1, 0:512], yps[0][D:D + 1, :])
            nc.vector.tensor_copy(stg[D:D + 1, 512:1024], yps[1][D:D + 1, :])
            lp = nrm.tile([1, 1024], f32, tag="lp", bufs=4,
                          name=f"lp_{c}_{j}")
            nc.gpsimd.dma_start(lp, stg[D:D + 1, :])
            lps[(c, j)] = lp
            nc.vector.tensor_copy(yt_sb[0:D, c, jsl], yps[0][0:D, :])
            tmpb = drn.tile([D, 512], bf16, tag="tmpb", bufs=3)
            nc.vector.tensor_copy(tmpb, yps[1][0:D, :])
            nc.gpsimd.dma_start(yt_sb[64:128, c, jsl], tmpb)
            if dbgL_d is not None:
                nc.sync.dma_start(dbgL_d[4 * j + c], stg[D:D + 1, :])

        def norm(c, j, eng=None):
            """Normalize yt tile (c, j): two partition_broadcasts of the
            rcp rows + two in-place multiplies. Emitted ~a block after the
            drain so every dep is satisfied on arrival."""
            if norm_mode == "skip":
                return
            eng = eng or nc.vector
            jsl = slice(j * 512, (j + 1) * 512)
            lp = lps.pop((c, j))
            rc = nrm.tile([1, 1024], f32, tag="rc", bufs=2)
            nc.vector.reciprocal_approx_fast(rc, lp)
            rcb = nrm.tile([1, 1024], bf16, tag="rcb", bufs=2)
            nc.vector.tensor_copy(rcb, rc)
            bc = nrm.tile([P, 1024], bf16, tag="bc", bufs=3)
            nc.gpsimd.partition_broadcast(bc[:, 0:512], rcb[0:1, 0:512])
            eng.tensor_tensor(
                yt_sb[0:D, c, jsl], yt_sb[0:D, c, jsl], bc[0:D, 0:512], MULT)
            nc.gpsimd.partition_broadcast(bc[:, 512:1024], rcb[0:1, 512:1024])
            if dbgB_d is not None:
                nc.sync.dma_start(dbgB_d[4 * j + c], rcb)
            eng.tensor_tensor(
                yt_sb[D:P, c, jsl], yt_sb[D:P, c, jsl],
                bc[D:P, 512:1024], MULT)

        # ------------------- emission schedule -------------------
        # j=0 starts as soon as quarter-0 V chains + pair-0 QK chains are
        # in; remaining projections and (later) out-proj row tiles are
        # popped as fillers between chunks, norm(c,j) one block after its
        # drain. All exp-pacing, PE never starved.
        for tti in range(4):
            piece_v(0, tti)
        # deferred x quarters 2-3: their triggers sit behind quarter-0's
        # affine_select in the gpsimd FIFO, so the transfers start only
        # once the head-critical DMAs are off the HBM bus
        nc.gpsimd.dma_start(x_sb[:, 2], xT_d[:, 2])
        nc.gpsimd.dma_start(x_sb[:, 3], xT_d[:, 3])
        piece_qk(0, 0)
        piece_qk(0, 4)

        F = []
        for ft in (1, 5, 2, 6, 3, 7):
            F.append(lambda ft=ft: piece_qk(0, ft))
        for tti in range(4):
            F.append(lambda tti=tti: piece_v(1, tti))

        att_block(0, 0, F, every=1)
        att_block(1, 0, F, every=1)
        att_block(2, 0, F, every=1)
        att_block(3, 0, F, every=1)
        while F:
            F.pop(0)()

        # j=1: fillers = quarter-2 V chains + T-half-1 QK chains (Q tiles
        # 2-3 + K quarters 2-3) + norms of tile 0
        F = [lambda: norm(0, 0)]
        for tti in range(4):
            F.append(lambda tti=tti: piece_v(2, tti))
        F.append(lambda: norm(1, 0))
        for ft in (0, 4, 1, 5):
            F.append(lambda ft=ft: piece_qk(1, ft))
        F.append(lambda: norm(2, 0))
        F.append(lambda: norm(3, 0))

        att_block(0, 1, F, every=2)
        att_block(1, 1, F, every=2)
        att_block(2, 1, F, every=2)
        att_block(3, 1, F, every=2)
        while F:
            F.pop(0)()

        # j=2: fillers = quarter-3 V chains + remaining QK + norms of tile 1
        F = [lambda: norm(0, 1)]
        for tti in range(4):
            F.append(lambda tti=tti: piece_v(3, tti))
        F.append(lambda: norm(1, 1))
        for ft in (2, 6, 3, 7):
            F.append(lambda ft=ft: piece_qk(1, ft))
        F.append(lambda: norm(2, 1))
        F.append(lambda: norm(3, 1))

        att_block(0, 2, F, every=2)
        att_block(1, 2, F, every=2)
        att_block(2, 2, F, every=2)
        att_block(3, 2, F, every=2)
        while F:
            F.pop(0)()

        # projections done: swap the proj PSUM banks for the out-proj pool
        psp_ctx.close()
        pools["pso"] = top.enter_context(
            tc.tile_pool(name="pso", bufs=2, space="PSUM"))

        # j=3: fillers = norms of tile 2 + out-proj of quarters 0-2
        F = [lambda: norm(0, 2)]
        F.append(lambda: piece_out(0))
        F.append(lambda: norm(1, 2))
        for tt in (1, 2):
            F.append(lambda tt=tt: piece_out(tt))
        F.append(lambda: norm(2, 2))
        for tt in (3, 4):
            F.append(lambda tt=tt: piece_out(tt))
        F.append(lambda: norm(3, 2))
        for tt in (5, 6, 7, 8):
            F.append(lambda tt=tt: piece_out(tt))

        att_block(0, 3, F, every=2)
        F.append(lambda: norm(0, 3))
        for tt in (9, 10):
            F.append(lambda tt=tt: piece_out(tt))
        att_block(1, 3, F, every=2)
        F.append(lambda: norm(1, 3))
        F.append(lambda: piece_out(11))
        att_block(2, 3, F, every=2)
        F.append(lambda: norm(2, 3))
        att_block(3, 3, F, every=2)
        while F:
            F.pop(0)()
        norm(3, 3)
        for tt in range(12, 16):
            piece_out(tt)



def _shard_inputs(x, w_qkv, w_out):
    mask = np.tril(np.ones((P, P), dtype=np.float32)).T
    mask = mask.astype(ml_dtypes.bfloat16)
    in_maps = []
    for core in range(8):
        b, hg = core // 2, core % 2
        sl = slice(hg * 512, (hg + 1) * 512)
        wq = w_qkv[0:1024][sl]
        wk = w_qkv[1024:2048][sl]
        wv = w_qkv[2048:3072][sl]
        wqkvT = np.concatenate([wq, wk, wv], axis=0).T  # [E, 1536]
        wvP = wqkvT[:, 1024:1536].reshape(NE, P, 512).transpose(1, 0, 2)
        wqkP = wqkvT[:, 0:1024].reshape(NE, P, 8, P).transpose(1, 2, 0, 3)
        # interleave the ft axis in attention need-order: 0,4,1,5,2,6,3,7
        wqkP = wqkP[:, (0, 4, 1, 5, 2, 6, 3, 7)]
        woT = w_out[:, sl].T  # [512, E]
        woP = woT.reshape(4, P, E).transpose(1, 0, 2)
        xT = (x[b].T.reshape(NE, P, 4, 512)
              .transpose(1, 2, 0, 3))  # [P, th, NE, 512]
        cvt = lambda a: np.ascontiguousarray(a).astype(ml_dtypes.bfloat16)
        in_maps.append({
            "xT": cvt(xT),
            "wvP": cvt(wvP),
            "wqkP": cvt(wqkP),
            "woP": cvt(woP),
            "mask": mask,
        })
    return in_maps


def kernel(x, w_qkv, w_out, _trace=False):
    x = np.asarray(x, dtype=np.float32)
    w_qkv = np.asarray(w_qkv, dtype=np.float32)
    w_out = np.asarray(w_out, dtype=np.float32)

    if "nc" not in _CACHE:
        _CACHE["nc"] = build()
    nc = _CACHE["nc"]

    in_maps = _shard_inputs(x, w_qkv, w_out)
    res = bass_utils.run_bass_kernel_spmd(
        nc, in_maps, core_ids=list(range(8)), trace=_trace)
    kernel.last_result = res

    out = np.empty((B, T, E), dtype=np.float32)
    for b in range(B):
        out[b] = (res.results[2 * b]["out"].astype(np.float32)
                  + res.results[2 * b + 1]["out"].astype(np.float32))
    return out
